# revision 9
# baseline (speedup 1.0000x reference)
"""Adagnn-with-weight GNN message-passing kernel for 8 Trainium2 NeuronCores.

Reference computation (N=100000 nodes, E=3200000 edges, F=256):
    e1  = segment_sum(edge_val[:,None] * x[edge_col], edge_row)   # spmm
    out = (x - e1 * (learnable_diag + 1)) @ weight + bias

Design notes (evidence from perfetto traces):
  - The wall is SWDGE descriptor generation for the per-edge gather
    (~2 ns/slot, serialized on the gpsimd engine).  Therefore: (a) host-side
    row->tile rebalancing minimizes pad slots (12.5% -> ~5.5%), (b) gathers
    are merged across a super-tile of ST dest tiles (fewer calls -> less
    fixed overhead), (c) nothing else may exceed ~900us.
  - Dest rows are assigned to (core, tile) by a greedy bin-packing that
    equalizes per-(tile, source-block) edge counts across cores (the chunk
    table is shared by all 8 cores).  Output rows are un-permuted on host.
  - DVE one-hot A build pays ~69ns per 128-elem AP row; the output/in0 APs
    are flattened to 2D to (attempt to) amortize it.
  - Epilogue in transposed space: e4T = e1T*(-dscaleT) + xoT fused on DVE,
    bias added via a contract-1 matmul, PSUM->SBUF copies on ACT.
"""

import numpy as np

import concourse.bacc as bacc
import concourse.mybir as mybir
import concourse.tile as tile
from concourse.bass_utils import run_bass_kernel_spmd

FP = mybir.dt.float32
BF = mybir.dt.bfloat16
BF_NP = mybir.dt.np(BF)


class Cfg:
    def __init__(self, n_nodes=100000, n_edges=3200000, f=256, n_cores=8,
                 nb=4, st=4, gather_bufs=7, amat_bufs=2, flat_a=True,
                 rebalance=True):
        self.N = n_nodes
        self.E = n_edges
        self.F = f
        self.NC = n_cores
        self.NB = nb
        self.ST = st
        self.RPC = n_nodes // n_cores
        self.TILES = (self.RPC + 127) // 128
        self.PAD_ROWS = self.TILES * 128
        self.NST = (self.TILES + st - 1) // st
        self.BLK = n_nodes // nb
        assert self.BLK < (1 << 15)
        self.gather_bufs = gather_bufs
        self.amat_bufs = amat_bufs
        self.flat_a = flat_a
        self.rebalance = rebalance


def _assign_rows(cfg, edge_row, edge_col):
    """Greedy bin-packing of dest rows into (core, tile) bins, minimizing
    per-(tile, block) overflow above 8 chunks.  Returns row_perm[N] giving
    the device row ordering: device row (c, t, d) holds original row
    row_perm[c*PAD_ROWS + t*128 + d] (or -1 for unused pad slots)."""
    N, NB, NC, TILES = cfg.N, cfg.NB, cfg.NC, cfg.TILES
    f = np.zeros((N, NB), dtype=np.int64)
    np.add.at(f, (edge_row, edge_col // cfg.BLK), 1)
    if not cfg.rebalance:
        gbin = np.zeros(N, dtype=np.int64)
        for c in range(NC):
            rr = np.arange(c * cfg.RPC, (c + 1) * cfg.RPC)
            gbin[rr] = c * TILES + np.minimum((rr - c * cfg.RPC) // 128,
                                              TILES - 1)
    else:
        tot = f.sum(1)
        nbins = NC * TILES
        capm = np.full((NC, TILES), 128)
        capm[:, -1] = cfg.RPC - (TILES - 1) * 128
        cap = capm.reshape(-1)
        load = np.zeros((nbins, NB), dtype=np.float64)
        cnt = np.zeros(nbins, dtype=np.int64)
        gbin = np.empty(N, dtype=np.int64)
        for r in np.argsort(-tot, kind='stable'):
            nl = load + f[r]
            scores = (np.maximum(nl - 1024.0, 0).sum(axis=1)
                      + nl.max(axis=1) * 1e-3 + (cnt >= cap) * 1e9)
            bb = int(np.argmin(scores))
            gbin[r] = bb
            load[bb] += f[r]
            cnt[bb] += 1

    # row_perm & per-row device slot
    order = np.argsort(gbin, kind='stable')
    row_perm = np.full(NC * cfg.PAD_ROWS, -1, dtype=np.int64)
    row_slot = np.empty(N, dtype=np.int64)  # device row id per original row
    pos_in_bin = np.zeros(NC * TILES, dtype=np.int64)
    gb_sorted = gbin[order]
    # rank within bin
    starts = np.searchsorted(gb_sorted, np.arange(NC * TILES), side='left')
    rank = np.arange(N) - starts[gb_sorted]
    c = gb_sorted // TILES
    t = gb_sorted % TILES
    dev = c * cfg.PAD_ROWS + t * 128 + rank
    row_perm[dev] = order
    row_slot[order] = dev
    return row_perm, row_slot


def _preprocess(cfg, edge_row, edge_col, edge_val, row_slot):
    """Partition + sort + pad the edge list using the device row mapping."""
    edge_row = np.asarray(edge_row).astype(np.int64)
    edge_col = np.asarray(edge_col).astype(np.int64)
    edge_val = np.asarray(edge_val).astype(np.float32)
    NC, TILES, NB, ST, NST, E = cfg.NC, cfg.TILES, cfg.NB, cfg.ST, cfg.NST, cfg.E

    dev = row_slot[edge_row]
    core = dev // cfg.PAD_ROWS
    dloc = dev - core * cfg.PAD_ROWS
    t = dloc >> 7
    d = (dloc & 127).astype(np.float32)
    b = edge_col // cfg.BLK
    cloc = (edge_col - b * cfg.BLK).astype(np.int16)

    # cell order: (super-tile, block, tile-in-st)
    ncell = TILES * NB
    cell_rank = np.zeros((TILES, NB), dtype=np.int64)
    rank = 0
    for stt in range(NST):
        t0, t1 = stt * ST, min((stt + 1) * ST, TILES)
        for bb in range(NB):
            for tt in range(t0, t1):
                cell_rank[tt, bb] = rank
                rank += 1
    assert rank == ncell

    key = core * ncell + cell_rank[t, b]
    order = np.lexsort((cloc, key))
    key_s = key[order]

    counts = np.bincount(key, minlength=NC * ncell).reshape(NC, ncell)
    C = np.ceil(counts.max(axis=0) / 128).astype(np.int64)
    pad_off = np.concatenate([[0], np.cumsum(128 * C)])
    L = int(pad_off[-1])

    starts = np.searchsorted(key_s, np.arange(NC * ncell), side="left")
    rnk = np.arange(E) - starts[key_s]
    pos = (key_s // ncell) * L + pad_off[key_s % ncell] + rnk

    col_pad = np.zeros(NC * L, dtype=np.int16)   # pads gather block row 0
    dest_pad = np.zeros(NC * L, dtype=np.float32)
    val_pad = np.zeros(NC * L, dtype=np.float32)
    col_pad[pos] = cloc[order]
    dest_pad[pos] = d[order]
    val_pad[pos] = edge_val[order]

    col_pad = col_pad.reshape(NC, L)
    dest_pad = dest_pad.reshape(NC, L)
    val_pad = val_pad.reshape(NC, L)

    idx_packed = np.tile(
        col_pad.reshape(NC, L // 16, 16).transpose(0, 2, 1), (1, 8, 1)
    )  # [NC, 128, L//16]

    n_t = np.array([sum(int(C[cell_rank[tt, bb]]) for bb in range(NB))
                    for tt in range(TILES)])
    CT = int(n_t.sum())
    dest_cols = np.zeros((NC, 128, CT), dtype=BF_NP)
    val_cols = np.zeros((NC, 128, CT), dtype=BF_NP)
    tile_coff = np.concatenate([[0], np.cumsum(n_t)])
    for tt in range(TILES):
        toff = int(tile_coff[tt])
        for bb in range(NB):
            r = cell_rank[tt, bb]
            o0, n = int(pad_off[r]), int(C[r])
            if n == 0:
                continue
            seg = slice(o0, o0 + 128 * n)
            dest_cols[:, :, toff:toff + n] = (
                dest_pad[:, seg].reshape(NC, n, 128).transpose(0, 2, 1))
            val_cols[:, :, toff:toff + n] = (
                val_pad[:, seg].reshape(NC, n, 128).transpose(0, 2, 1))
            toff += n

    tables = dict(C=C, cell_rank=cell_rank, pad_off=pad_off, n_t=n_t,
                  tile_coff=tile_coff)
    return tables, idx_packed, dest_cols, val_cols


def _build(cfg, tables):
    F, NB, ST, NST, TILES = cfg.F, cfg.NB, cfg.ST, cfg.NST, cfg.TILES
    C = tables["C"]
    cell_rank = tables["cell_rank"]
    pad_off = tables["pad_off"]
    n_t = tables["n_t"]
    tile_coff = tables["tile_coff"]
    L = int(pad_off[-1])
    C_MAXT = int(n_t.max())
    KC = F // 128

    C_stb = np.zeros((NST, NB), dtype=np.int64)
    gcall_off = np.zeros((NST, NB), dtype=np.int64)
    for stt in range(NST):
        t0, t1 = stt * ST, min((stt + 1) * ST, TILES)
        for bb in range(NB):
            gcall_off[stt, bb] = pad_off[cell_rank[t0, bb]] // 128
            C_stb[stt, bb] = sum(int(C[cell_rank[tt, bb]])
                                 for tt in range(t0, t1))
    C_MAXG = int(C_stb.max())

    nc = bacc.Bacc("TRN2", target_bir_lowering=False, debug=False,
                   num_swdge_queues=4)

    xsrc = nc.dram_tensor("xsrc", [cfg.N, F], BF, kind="ExternalInput")
    xot_d = nc.dram_tensor("xot", [128, TILES, KC, 128], BF,
                           kind="ExternalInput")
    idx_d = nc.dram_tensor("idx", [128, L // 16], mybir.dt.int16,
                           kind="ExternalInput")
    dest_d = nc.dram_tensor("dest", [128, int(n_t.sum())], BF,
                            kind="ExternalInput")
    val_d = nc.dram_tensor("val", [128, int(n_t.sum())], BF,
                           kind="ExternalInput")
    w_d = nc.dram_tensor("wt", [128, KC, F], BF, kind="ExternalInput")
    negds_d = nc.dram_tensor("negds", [128, KC], FP, kind="ExternalInput")
    biasrow_d = nc.dram_tensor("biasrow", [1, F], BF, kind="ExternalInput")
    iota_d = nc.dram_tensor("iota", [128, 128], BF, kind="ExternalInput")
    ident_d = nc.dram_tensor("ident", [128, 128], BF, kind="ExternalInput")
    zeros_d = nc.dram_tensor("zeros", [128, 1], FP, kind="ExternalInput")
    out_d = nc.dram_tensor("out", [cfg.PAD_ROWS, F], BF,
                           kind="ExternalOutput")

    with tile.TileContext(nc) as tc:
        with (
            tc.tile_pool(name="const", bufs=1) as cpool,
            tc.tile_pool(name="gather", bufs=cfg.gather_bufs) as gpool,
            tc.tile_pool(name="amat", bufs=cfg.amat_bufs) as apool,
            tc.tile_pool(name="meta", bufs=8) as mpool,
            tc.tile_pool(name="work", bufs=4) as wpool,
            tc.tile_pool(name="pse1", bufs=2, space="PSUM") as e1pool,
            tc.tile_pool(name="pstr", bufs=2, space="PSUM") as trpool,
            tc.tile_pool(name="psout", bufs=2, space="PSUM") as opool,
        ):
            w_t = cpool.tile([128, KC, F], BF)
            negds_t = cpool.tile([128, KC], FP)
            biasrow_t = cpool.tile([1, F], BF)
            iota_t = cpool.tile([128, 128], BF)
            ident_t = cpool.tile([128, 128], BF)
            zeros_t = cpool.tile([128, 1], FP)
            ones_t = cpool.tile([1, 128], BF)
            nc.sync.dma_start(w_t[:], w_d[:])
            nc.sync.dma_start(negds_t[:], negds_d[:])
            nc.sync.dma_start(biasrow_t[:], biasrow_d[:])
            nc.sync.dma_start(iota_t[:], iota_d[:])
            nc.sync.dma_start(ident_t[:], ident_d[:])
            nc.sync.dma_start(zeros_t[:], zeros_d[:])
            nc.vector.memset(ones_t[:], 1.0)

            iota_rep = cpool.tile([128, C_MAXT, 128], BF)
            nc.vector.tensor_copy(
                iota_rep[:], iota_t[:, None, :].broadcast_to((128, C_MAXT, 128)))

            xgs_by_st = {}
            tile_state = {}
            pending = [None]

            def stage_a(stt, tt):
                """meta DMAs + one-hot A build for tile tt."""
                nt = int(n_t[tt])
                toff = int(tile_coff[tt])
                dest_t = mpool.tile([128, C_MAXT], BF, tag="dest")
                nc.sync.dma_start(dest_t[:, :nt], dest_d[:, toff:toff + nt])
                val_t = mpool.tile([128, C_MAXT], BF, tag="val")
                nc.sync.dma_start(val_t[:, :nt], val_d[:, toff:toff + nt])
                xot = wpool.tile([128, KC, 128], BF, tag="xot")
                nc.sync.dma_start(xot[:], xot_d[:, tt])

                a_t = apool.tile([128, C_MAXT, 128], BF, tag="a")
                dest_b = dest_t[:, :nt, None].broadcast_to((128, nt, 128))
                val_b = val_t[:, :nt, None].broadcast_to((128, nt, 128))
                a_f = a_t.rearrange("p a b -> p (a b)")[:, :nt * 128]
                iota_f = iota_rep.rearrange("p a b -> p (a b)")[:, :nt * 128]
                nc.vector.tensor_tensor(a_f, iota_f, dest_b,
                                        op=mybir.AluOpType.is_equal)
                nc.vector.tensor_tensor(a_f, a_f, val_b,
                                        op=mybir.AluOpType.mult)
                tile_state[tt] = (a_t, xot)

            def stage_b(stt, tt):
                """segment-sum matmuls + epilogue for tile tt."""
                nt = int(n_t[tt])
                a_t, xot = tile_state.pop(tt)
                xgs = xgs_by_st[stt]
                e1 = e1pool.tile([128, F], FP, tag="e1")
                cc = 0
                for bb in range(NB):
                    cb = int(C[cell_rank[tt, bb]])
                    if cb == 0:
                        continue
                    base = int((pad_off[cell_rank[tt, bb]] // 128)
                               - gcall_off[stt, bb])
                    xg = xgs[bb]
                    for c in range(cb):
                        nc.tensor.matmul(
                            e1[:], a_t[:, cc, :], xg[:, base + c, :],
                            start=(cc == 0), stop=(cc == nt - 1),
                        )
                        cc += 1

                e1_sb = wpool.tile([128, F], BF, tag="e1sb")
                nc.scalar.activation(e1_sb[:], e1[:],
                                     mybir.ActivationFunctionType.Identity,
                                     bias=zeros_t[:])
                e1T = trpool.tile([128, KC, 128], BF, tag="tr")
                for kc in range(KC):
                    nc.tensor.transpose(e1T[:, kc, :],
                                        e1_sb[:, kc * 128:(kc + 1) * 128],
                                        ident_t[:])

                e4T = wpool.tile([128, KC, 128], BF, tag="e4T")
                for kc in range(KC):
                    nc.vector.scalar_tensor_tensor(
                        e4T[:, kc, :], e1T[:, kc, :],
                        negds_t[:, kc:kc + 1], xot[:, kc, :],
                        op0=mybir.AluOpType.mult,
                        op1=mybir.AluOpType.add)

                ps_out = opool.tile([128, F], FP, tag="po")
                for kc in range(KC):
                    nc.tensor.matmul(ps_out[:], e4T[:, kc, :], w_t[:, kc, :],
                                     start=(kc == 0), stop=False)
                nc.tensor.matmul(ps_out[:], ones_t[:], biasrow_t[:],
                                 start=False, stop=True)
                outs = wpool.tile([128, F], BF, tag="outs")
                nc.scalar.activation(outs[:], ps_out[:],
                                     mybir.ActivationFunctionType.Identity,
                                     bias=zeros_t[:])
                nc.sync.dma_start(out_d[tt * 128:(tt + 1) * 128, :], outs[:])

            for stt in range(NST):
                t0, t1 = stt * ST, min((stt + 1) * ST, TILES)

                xgs = {}
                for bb in range(NB):
                    cg = int(C_stb[stt, bb])
                    if cg == 0:
                        continue
                    o16 = int(gcall_off[stt, bb]) * 8
                    idx_t = mpool.tile([128, 8 * C_MAXG], mybir.dt.int16,
                                       tag="idx")
                    # only partitions [32b, 32b+32) are read by queue b's
                    # Q7 pair; skip the other 3 replicas
                    nc.sync.dma_start(idx_t[32 * bb:32 * bb + 32, :8 * cg],
                                      idx_d[32 * bb:32 * bb + 32,
                                            o16:o16 + 8 * cg])
                    xg = gpool.tile([128, C_MAXG, F], BF, tag="xg")
                    nc.gpsimd.dma_gather(
                        xg[:, :cg, :],
                        xsrc[bb * cfg.BLK:(bb + 1) * cfg.BLK, :],
                        idx_t[:, :8 * cg],
                        num_idxs=128 * cg,
                        num_idxs_reg=128 * cg,
                        elem_size=F,
                        single_packet=False,
                        queue_num=bb,
                    )
                    xgs[bb] = xg
                xgs_by_st[stt] = xgs

                for tt in range(t0, t1):
                    stage_a(stt, tt)
                    if pending[0] is not None:
                        stage_b(*pending[0])
                    pending[0] = (stt, tt)
            if pending[0] is not None:
                stage_b(*pending[0])

    nc.compile()
    return nc


def _make_in_maps(cfg, x, weight, learnable_diag, bias, row_perm,
                  idx_packed, dest_cols, val_cols):
    F, NC, TILES = cfg.F, cfg.NC, cfg.TILES
    KC = F // 128
    x16 = x.astype(BF_NP)
    w_host = np.ascontiguousarray(
        weight.reshape(KC, 128, F).transpose(1, 0, 2)).astype(BF_NP)
    negds_host = np.ascontiguousarray(
        -(learnable_diag + 1.0).reshape(KC, 128).T).astype(np.float32)
    biasrow_host = bias.reshape(1, F).astype(BF_NP)
    iota_host = np.tile(np.arange(128, dtype=np.float32)[None, :],
                        (128, 1)).astype(BF_NP)
    ident_host = np.eye(128, dtype=np.float32).astype(BF_NP)
    zeros_host = np.zeros((128, 1), dtype=np.float32)

    # residual x rows in device order (permuted), transposed per tile:
    # xot[p, t, kc, d] = xperm[t*128 + d, kc*128 + p]
    xperm = np.zeros((NC * cfg.PAD_ROWS, F), dtype=np.float32)
    valid = row_perm >= 0
    xperm[valid] = x[row_perm[valid]]
    xot_host = np.ascontiguousarray(
        xperm.reshape(NC, TILES, 128, KC, 128).transpose(0, 4, 1, 3, 2)
    ).astype(BF_NP)

    in_maps = []
    for c in range(NC):
        in_maps.append({
            "xsrc": x16,
            "xot": xot_host[c],
            "idx": np.ascontiguousarray(idx_packed[c]),
            "dest": np.ascontiguousarray(dest_cols[c]),
            "val": np.ascontiguousarray(val_cols[c]),
            "wt": w_host,
            "negds": negds_host,
            "biasrow": biasrow_host,
            "iota": iota_host,
            "ident": ident_host,
            "zeros": zeros_host,
        })
    return in_maps


def run(cfg, x, edge_row, edge_col, edge_val, weight, learnable_diag, bias,
        trace_dir=None):
    x = np.ascontiguousarray(np.asarray(x, dtype=np.float32))
    edge_row = np.asarray(edge_row).astype(np.int64)
    edge_col = np.asarray(edge_col).astype(np.int64)
    weight = np.asarray(weight, dtype=np.float32)
    learnable_diag = np.asarray(learnable_diag, dtype=np.float32)
    bias = np.asarray(bias, dtype=np.float32)

    row_perm, row_slot = _assign_rows(cfg, edge_row, edge_col)
    tables, idx_packed, dest_cols, val_cols = _preprocess(
        cfg, edge_row, edge_col, edge_val, row_slot)
    nc = _build(cfg, tables)
    in_maps = _make_in_maps(cfg, x, weight, learnable_diag, bias, row_perm,
                            idx_packed, dest_cols, val_cols)

    kwargs = {}
    if trace_dir:
        kwargs = dict(trace=True, tmpdir=trace_dir)
    res = run_bass_kernel_spmd(nc, in_maps, core_ids=list(range(cfg.NC)),
                               **kwargs)
    out = np.empty((cfg.N, cfg.F), dtype=np.float32)
    for c in range(cfg.NC):
        dev_rows = res.results[c]["out"].astype(np.float32)
        rp = row_perm[c * cfg.PAD_ROWS:(c + 1) * cfg.PAD_ROWS]
        m = rp >= 0
        out[rp[m]] = dev_rows[m]
    return out, res


def kernel(x, edge_row, edge_col, edge_val, weight, learnable_diag, bias,
           _want_trace=None):
    cfg = Cfg()
    out, res = run(cfg, x, edge_row, edge_col, edge_val, weight,
                   learnable_diag, bias, trace_dir=_want_trace)
    kernel._last_results = res
    return out


# revision 10
# speedup vs baseline: 1.1212x; 1.1212x over previous
"""Adagnn-with-weight GNN message-passing kernel for 8 Trainium2 NeuronCores.

Reference computation (N=100000 nodes, E=3200000 edges, F=256):
    e1  = segment_sum(edge_val[:,None] * x[edge_col], edge_row)   # spmm
    out = (x - e1 * (learnable_diag + 1)) @ weight + bias

Design notes (evidence from perfetto traces):
  - The wall is SWDGE descriptor generation for the per-edge gather
    (~2 ns/slot, serialized on the gpsimd engine).  Therefore: (a) host-side
    row->tile rebalancing minimizes pad slots (12.5% -> ~5.5%), (b) gathers
    are merged across a super-tile of ST dest tiles (fewer calls -> less
    fixed overhead), (c) nothing else may exceed ~900us.
  - Dest rows are assigned to (core, tile) by a greedy bin-packing that
    equalizes per-(tile, source-block) edge counts across cores (the chunk
    table is shared by all 8 cores).  Output rows are un-permuted on host.
  - DVE one-hot A build pays ~69ns per 128-elem AP row; the output/in0 APs
    are flattened to 2D to (attempt to) amortize it.
  - Epilogue in transposed space: e4T = e1T*(-dscaleT) + xoT fused on DVE,
    bias added via a contract-1 matmul, PSUM->SBUF copies on ACT.
"""

import numpy as np

import concourse.bacc as bacc
import concourse.mybir as mybir
import concourse.tile as tile
from concourse.bass_utils import run_bass_kernel_spmd

FP = mybir.dt.float32
BF = mybir.dt.bfloat16
BF_NP = mybir.dt.np(BF)


class Cfg:
    def __init__(self, n_nodes=100000, n_edges=3200000, f=256, n_cores=8,
                 nb=4, st=4, gather_bufs=7, amat_bufs=2, flat_a=True,
                 rebalance=True):
        self.N = n_nodes
        self.E = n_edges
        self.F = f
        self.NC = n_cores
        self.NB = nb
        self.ST = st
        self.RPC = n_nodes // n_cores
        self.TILES = (self.RPC + 127) // 128
        self.PAD_ROWS = self.TILES * 128
        self.NST = (self.TILES + st - 1) // st
        self.BLK = n_nodes // nb
        assert self.BLK < (1 << 15)
        self.gather_bufs = gather_bufs
        self.amat_bufs = amat_bufs
        self.flat_a = flat_a
        self.rebalance = rebalance


def _assign_rows(cfg, edge_row, edge_col):
    """Greedy bin-packing of dest rows into (core, tile) bins, minimizing
    per-(tile, block) overflow above 8 chunks.  Returns row_perm[N] giving
    the device row ordering: device row (c, t, d) holds original row
    row_perm[c*PAD_ROWS + t*128 + d] (or -1 for unused pad slots)."""
    N, NB, NC, TILES = cfg.N, cfg.NB, cfg.NC, cfg.TILES
    f = np.zeros((N, NB), dtype=np.int64)
    np.add.at(f, (edge_row, edge_col // cfg.BLK), 1)
    if not cfg.rebalance:
        gbin = np.zeros(N, dtype=np.int64)
        for c in range(NC):
            rr = np.arange(c * cfg.RPC, (c + 1) * cfg.RPC)
            gbin[rr] = c * TILES + np.minimum((rr - c * cfg.RPC) // 128,
                                              TILES - 1)
    else:
        tot = f.sum(1)
        nbins = NC * TILES
        capm = np.full((NC, TILES), 128)
        capm[:, -1] = cfg.RPC - (TILES - 1) * 128
        cap = capm.reshape(-1)
        load = np.zeros((nbins, NB), dtype=np.float64)
        cnt = np.zeros(nbins, dtype=np.int64)
        gbin = np.empty(N, dtype=np.int64)
        for r in np.argsort(-tot, kind='stable'):
            nl = load + f[r]
            scores = (np.maximum(nl - 1024.0, 0).sum(axis=1)
                      + nl.max(axis=1) * 1e-3 + (cnt >= cap) * 1e9)
            bb = int(np.argmin(scores))
            gbin[r] = bb
            load[bb] += f[r]
            cnt[bb] += 1

    # row_perm & per-row device slot
    order = np.argsort(gbin, kind='stable')
    row_perm = np.full(NC * cfg.PAD_ROWS, -1, dtype=np.int64)
    row_slot = np.empty(N, dtype=np.int64)  # device row id per original row
    pos_in_bin = np.zeros(NC * TILES, dtype=np.int64)
    gb_sorted = gbin[order]
    # rank within bin
    starts = np.searchsorted(gb_sorted, np.arange(NC * TILES), side='left')
    rank = np.arange(N) - starts[gb_sorted]
    c = gb_sorted // TILES
    t = gb_sorted % TILES
    dev = c * cfg.PAD_ROWS + t * 128 + rank
    row_perm[dev] = order
    row_slot[order] = dev
    return row_perm, row_slot


def _preprocess(cfg, edge_row, edge_col, edge_val, row_slot):
    """Partition + sort + pad the edge list using the device row mapping."""
    edge_row = np.asarray(edge_row).astype(np.int64)
    edge_col = np.asarray(edge_col).astype(np.int64)
    edge_val = np.asarray(edge_val).astype(np.float32)
    NC, TILES, NB, ST, NST, E = cfg.NC, cfg.TILES, cfg.NB, cfg.ST, cfg.NST, cfg.E

    dev = row_slot[edge_row]
    core = dev // cfg.PAD_ROWS
    dloc = dev - core * cfg.PAD_ROWS
    t = dloc >> 7
    d = (dloc & 127).astype(np.float32)
    b = edge_col // cfg.BLK
    cloc = (edge_col - b * cfg.BLK).astype(np.int16)

    # cell order: (super-tile, block, tile-in-st)
    ncell = TILES * NB
    cell_rank = np.zeros((TILES, NB), dtype=np.int64)
    rank = 0
    for stt in range(NST):
        t0, t1 = stt * ST, min((stt + 1) * ST, TILES)
        for bb in range(NB):
            for tt in range(t0, t1):
                cell_rank[tt, bb] = rank
                rank += 1
    assert rank == ncell

    key = core * ncell + cell_rank[t, b]
    order = np.lexsort((cloc, key))
    key_s = key[order]

    counts = np.bincount(key, minlength=NC * ncell).reshape(NC, ncell)
    C = np.ceil(counts.max(axis=0) / 128).astype(np.int64)
    pad_off = np.concatenate([[0], np.cumsum(128 * C)])
    L = int(pad_off[-1])

    starts = np.searchsorted(key_s, np.arange(NC * ncell), side="left")
    rnk = np.arange(E) - starts[key_s]
    pos = (key_s // ncell) * L + pad_off[key_s % ncell] + rnk

    col_pad = np.zeros(NC * L, dtype=np.int16)   # pads gather block row 0
    dest_pad = np.zeros(NC * L, dtype=np.float32)
    val_pad = np.zeros(NC * L, dtype=np.float32)
    col_pad[pos] = cloc[order]
    dest_pad[pos] = d[order]
    val_pad[pos] = edge_val[order]

    col_pad = col_pad.reshape(NC, L)
    dest_pad = dest_pad.reshape(NC, L)
    val_pad = val_pad.reshape(NC, L)

    idx_packed = np.tile(
        col_pad.reshape(NC, L // 16, 16).transpose(0, 2, 1), (1, 8, 1)
    )  # [NC, 128, L//16]

    n_t = np.array([sum(int(C[cell_rank[tt, bb]]) for bb in range(NB))
                    for tt in range(TILES)])
    CT = int(n_t.sum())
    dest_cols = np.zeros((NC, 128, CT), dtype=BF_NP)
    val_cols = np.zeros((NC, 128, CT), dtype=BF_NP)
    tile_coff = np.concatenate([[0], np.cumsum(n_t)])
    for tt in range(TILES):
        toff = int(tile_coff[tt])
        for bb in range(NB):
            r = cell_rank[tt, bb]
            o0, n = int(pad_off[r]), int(C[r])
            if n == 0:
                continue
            seg = slice(o0, o0 + 128 * n)
            dest_cols[:, :, toff:toff + n] = (
                dest_pad[:, seg].reshape(NC, n, 128).transpose(0, 2, 1))
            val_cols[:, :, toff:toff + n] = (
                val_pad[:, seg].reshape(NC, n, 128).transpose(0, 2, 1))
            toff += n

    tables = dict(C=C, cell_rank=cell_rank, pad_off=pad_off, n_t=n_t,
                  tile_coff=tile_coff)
    return tables, idx_packed, dest_cols, val_cols


def _build(cfg, tables):
    F, NB, ST, NST, TILES = cfg.F, cfg.NB, cfg.ST, cfg.NST, cfg.TILES
    C = tables["C"]
    cell_rank = tables["cell_rank"]
    pad_off = tables["pad_off"]
    n_t = tables["n_t"]
    tile_coff = tables["tile_coff"]
    L = int(pad_off[-1])
    C_MAXT = int(n_t.max())
    KC = F // 128

    C_stb = np.zeros((NST, NB), dtype=np.int64)
    gcall_off = np.zeros((NST, NB), dtype=np.int64)
    for stt in range(NST):
        t0, t1 = stt * ST, min((stt + 1) * ST, TILES)
        for bb in range(NB):
            gcall_off[stt, bb] = pad_off[cell_rank[t0, bb]] // 128
            C_stb[stt, bb] = sum(int(C[cell_rank[tt, bb]])
                                 for tt in range(t0, t1))
    C_MAXG = int(C_stb.max())

    nc = bacc.Bacc("TRN2", target_bir_lowering=False, debug=False,
                   num_swdge_queues=4)

    xsrc = nc.dram_tensor("xsrc", [cfg.N, F], BF, kind="ExternalInput")
    xot_d = nc.dram_tensor("xot", [128, TILES, KC, 128], BF,
                           kind="ExternalInput")
    idx_d = nc.dram_tensor("idx", [128, L // 16], mybir.dt.int16,
                           kind="ExternalInput")
    dest_d = nc.dram_tensor("dest", [128, int(n_t.sum())], BF,
                            kind="ExternalInput")
    val_d = nc.dram_tensor("val", [128, int(n_t.sum())], BF,
                           kind="ExternalInput")
    w_d = nc.dram_tensor("wt", [128, KC, F], BF, kind="ExternalInput")
    negds_d = nc.dram_tensor("negds", [128, KC], FP, kind="ExternalInput")
    biasrow_d = nc.dram_tensor("biasrow", [1, F], BF, kind="ExternalInput")
    iota_d = nc.dram_tensor("iota", [128, 128], BF, kind="ExternalInput")
    ident_d = nc.dram_tensor("ident", [128, 128], BF, kind="ExternalInput")
    zeros_d = nc.dram_tensor("zeros", [128, 1], FP, kind="ExternalInput")
    out_d = nc.dram_tensor("out", [cfg.PAD_ROWS, F], BF,
                           kind="ExternalOutput")

    with tile.TileContext(nc) as tc:
        with (
            tc.tile_pool(name="const", bufs=1) as cpool,
            tc.tile_pool(name="gather", bufs=cfg.gather_bufs) as gpool,
            tc.tile_pool(name="amat", bufs=cfg.amat_bufs) as apool,
            tc.tile_pool(name="meta", bufs=8) as mpool,
            tc.tile_pool(name="work", bufs=4) as wpool,
            tc.tile_pool(name="pse1", bufs=2, space="PSUM") as e1pool,
            tc.tile_pool(name="pstr", bufs=2, space="PSUM") as trpool,
            tc.tile_pool(name="psout", bufs=2, space="PSUM") as opool,
        ):
            w_t = cpool.tile([128, KC, F], BF)
            negds_t = cpool.tile([128, KC], FP)
            biasrow_t = cpool.tile([1, F], BF)
            iota_t = cpool.tile([128, 128], BF)
            ident_t = cpool.tile([128, 128], BF)
            zeros_t = cpool.tile([128, 1], FP)
            ones_t = cpool.tile([1, 128], BF)
            nc.sync.dma_start(w_t[:], w_d[:])
            nc.sync.dma_start(negds_t[:], negds_d[:])
            nc.sync.dma_start(biasrow_t[:], biasrow_d[:])
            nc.sync.dma_start(iota_t[:], iota_d[:])
            nc.sync.dma_start(ident_t[:], ident_d[:])
            nc.sync.dma_start(zeros_t[:], zeros_d[:])
            nc.vector.memset(ones_t[:], 1.0)

            iota_rep = cpool.tile([128, C_MAXT, 128], BF)
            nc.vector.tensor_copy(
                iota_rep[:], iota_t[:, None, :].broadcast_to((128, C_MAXT, 128)))

            xgs_by_st = {}
            tile_state = {}
            pending = [None]

            def stage_a(stt, tt):
                """meta DMAs + one-hot A build for tile tt."""
                nt = int(n_t[tt])
                toff = int(tile_coff[tt])
                dest_t = mpool.tile([128, C_MAXT], BF, tag="dest")
                nc.sync.dma_start(dest_t[:, :nt], dest_d[:, toff:toff + nt])
                val_t = mpool.tile([128, C_MAXT], BF, tag="val")
                nc.sync.dma_start(val_t[:, :nt], val_d[:, toff:toff + nt])
                xot = wpool.tile([128, KC, 128], BF, tag="xot")
                nc.sync.dma_start(xot[:], xot_d[:, tt])

                a_t = apool.tile([128, C_MAXT, 128], BF, tag="a")
                dest_b = dest_t[:, :nt, None].broadcast_to((128, nt, 128))
                val_b = val_t[:, :nt, None].broadcast_to((128, nt, 128))
                a_f = a_t.rearrange("p a b -> p (a b)")[:, :nt * 128]
                iota_f = iota_rep.rearrange("p a b -> p (a b)")[:, :nt * 128]
                nc.vector.tensor_tensor(a_f, iota_f, dest_b,
                                        op=mybir.AluOpType.is_equal)
                nc.vector.tensor_tensor(a_f, a_f, val_b,
                                        op=mybir.AluOpType.mult)
                tile_state[tt] = (a_t, xot)

            def stage_b(stt, tt):
                """segment-sum matmuls + epilogue for tile tt."""
                nt = int(n_t[tt])
                a_t, xot = tile_state.pop(tt)
                xgs = xgs_by_st[stt]
                e1 = e1pool.tile([128, F], FP, tag="e1")
                cc = 0
                for bb in range(NB):
                    cb = int(C[cell_rank[tt, bb]])
                    if cb == 0:
                        continue
                    base = int((pad_off[cell_rank[tt, bb]] // 128)
                               - gcall_off[stt, bb])
                    xg = xgs[bb]
                    for c in range(cb):
                        nc.tensor.matmul(
                            e1[:], a_t[:, cc, :], xg[:, base + c, :],
                            start=(cc == 0), stop=(cc == nt - 1),
                        )
                        cc += 1

                e1_sb = wpool.tile([128, F], BF, tag="e1sb")
                nc.scalar.activation(e1_sb[:], e1[:],
                                     mybir.ActivationFunctionType.Identity,
                                     bias=zeros_t[:])
                e1T = trpool.tile([128, KC, 128], BF, tag="tr")
                for kc in range(KC):
                    nc.tensor.transpose(e1T[:, kc, :],
                                        e1_sb[:, kc * 128:(kc + 1) * 128],
                                        ident_t[:])

                e4T = wpool.tile([128, KC, 128], BF, tag="e4T")
                for kc in range(KC):
                    nc.vector.scalar_tensor_tensor(
                        e4T[:, kc, :], e1T[:, kc, :],
                        negds_t[:, kc:kc + 1], xot[:, kc, :],
                        op0=mybir.AluOpType.mult,
                        op1=mybir.AluOpType.add)

                ps_out = opool.tile([128, F], FP, tag="po")
                for kc in range(KC):
                    nc.tensor.matmul(ps_out[:], e4T[:, kc, :], w_t[:, kc, :],
                                     start=(kc == 0), stop=False)
                nc.tensor.matmul(ps_out[:], ones_t[:], biasrow_t[:],
                                 start=False, stop=True)
                outs = wpool.tile([128, F], BF, tag="outs")
                nc.scalar.activation(outs[:], ps_out[:],
                                     mybir.ActivationFunctionType.Identity,
                                     bias=zeros_t[:])
                # out-DMA from ACT's HWDGE queue: keeps the sync queue free
                # for the (latency-critical) idx/meta loads
                nc.scalar.dma_start(out_d[tt * 128:(tt + 1) * 128, :],
                                    outs[:])

            for stt in range(NST):
                t0, t1 = stt * ST, min((stt + 1) * ST, TILES)

                xgs = {}
                for bb in range(NB):
                    cg = int(C_stb[stt, bb])
                    if cg == 0:
                        continue
                    o16 = int(gcall_off[stt, bb]) * 8
                    idx_t = mpool.tile([128, 8 * C_MAXG], mybir.dt.int16,
                                       tag="idx")
                    # only partitions [32b, 32b+32) are read by queue b's
                    # Q7 pair; skip the other 3 replicas
                    nc.sync.dma_start(idx_t[32 * bb:32 * bb + 32, :8 * cg],
                                      idx_d[32 * bb:32 * bb + 32,
                                            o16:o16 + 8 * cg])
                    xg = gpool.tile([128, C_MAXG, F], BF, tag="xg")
                    nc.gpsimd.dma_gather(
                        xg[:, :cg, :],
                        xsrc[bb * cfg.BLK:(bb + 1) * cfg.BLK, :],
                        idx_t[:, :8 * cg],
                        num_idxs=128 * cg,
                        num_idxs_reg=128 * cg,
                        elem_size=F,
                        single_packet=False,
                        queue_num=bb,
                    )
                    xgs[bb] = xg
                xgs_by_st[stt] = xgs

                for tt in range(t0, t1):
                    stage_a(stt, tt)
                    if pending[0] is not None:
                        stage_b(*pending[0])
                    pending[0] = (stt, tt)
            if pending[0] is not None:
                stage_b(*pending[0])

    nc.compile()
    return nc


def _make_in_maps(cfg, x, weight, learnable_diag, bias, row_perm,
                  idx_packed, dest_cols, val_cols):
    F, NC, TILES = cfg.F, cfg.NC, cfg.TILES
    KC = F // 128
    x16 = x.astype(BF_NP)
    w_host = np.ascontiguousarray(
        weight.reshape(KC, 128, F).transpose(1, 0, 2)).astype(BF_NP)
    negds_host = np.ascontiguousarray(
        -(learnable_diag + 1.0).reshape(KC, 128).T).astype(np.float32)
    biasrow_host = bias.reshape(1, F).astype(BF_NP)
    iota_host = np.tile(np.arange(128, dtype=np.float32)[None, :],
                        (128, 1)).astype(BF_NP)
    ident_host = np.eye(128, dtype=np.float32).astype(BF_NP)
    zeros_host = np.zeros((128, 1), dtype=np.float32)

    # residual x rows in device order (permuted), transposed per tile:
    # xot[p, t, kc, d] = xperm[t*128 + d, kc*128 + p]
    xperm = np.zeros((NC * cfg.PAD_ROWS, F), dtype=np.float32)
    valid = row_perm >= 0
    xperm[valid] = x[row_perm[valid]]
    xot_host = np.ascontiguousarray(
        xperm.reshape(NC, TILES, 128, KC, 128).transpose(0, 4, 1, 3, 2)
    ).astype(BF_NP)

    in_maps = []
    for c in range(NC):
        in_maps.append({
            "xsrc": x16,
            "xot": xot_host[c],
            "idx": np.ascontiguousarray(idx_packed[c]),
            "dest": np.ascontiguousarray(dest_cols[c]),
            "val": np.ascontiguousarray(val_cols[c]),
            "wt": w_host,
            "negds": negds_host,
            "biasrow": biasrow_host,
            "iota": iota_host,
            "ident": ident_host,
            "zeros": zeros_host,
        })
    return in_maps


def run(cfg, x, edge_row, edge_col, edge_val, weight, learnable_diag, bias,
        trace_dir=None):
    x = np.ascontiguousarray(np.asarray(x, dtype=np.float32))
    edge_row = np.asarray(edge_row).astype(np.int64)
    edge_col = np.asarray(edge_col).astype(np.int64)
    weight = np.asarray(weight, dtype=np.float32)
    learnable_diag = np.asarray(learnable_diag, dtype=np.float32)
    bias = np.asarray(bias, dtype=np.float32)

    row_perm, row_slot = _assign_rows(cfg, edge_row, edge_col)
    tables, idx_packed, dest_cols, val_cols = _preprocess(
        cfg, edge_row, edge_col, edge_val, row_slot)
    nc = _build(cfg, tables)
    in_maps = _make_in_maps(cfg, x, weight, learnable_diag, bias, row_perm,
                            idx_packed, dest_cols, val_cols)

    kwargs = {}
    if trace_dir:
        kwargs = dict(trace=True, tmpdir=trace_dir)
    res = run_bass_kernel_spmd(nc, in_maps, core_ids=list(range(cfg.NC)),
                               **kwargs)
    out = np.empty((cfg.N, cfg.F), dtype=np.float32)
    for c in range(cfg.NC):
        dev_rows = res.results[c]["out"].astype(np.float32)
        rp = row_perm[c * cfg.PAD_ROWS:(c + 1) * cfg.PAD_ROWS]
        m = rp >= 0
        out[rp[m]] = dev_rows[m]
    return out, res


def kernel(x, edge_row, edge_col, edge_val, weight, learnable_diag, bias,
           _want_trace=None):
    cfg = Cfg()
    out, res = run(cfg, x, edge_row, edge_col, edge_val, weight,
                   learnable_diag, bias, trace_dir=_want_trace)
    kernel._last_results = res
    return out


# revision 11
# speedup vs baseline: 1.2179x; 1.0862x over previous
"""Adagnn-with-weight GNN message-passing kernel for 8 Trainium2 NeuronCores.

Reference computation (N=100000 nodes, E=3200000 edges, F=256):
    e1  = segment_sum(edge_val[:,None] * x[edge_col], edge_row)   # spmm
    out = (x - e1 * (learnable_diag + 1)) @ weight + bias

Design notes (evidence from perfetto traces):
  - The wall is SWDGE descriptor generation for the per-edge gather
    (~2 ns/slot, serialized on the gpsimd engine).  Therefore: (a) host-side
    row->tile rebalancing minimizes pad slots (12.5% -> ~5.5%), (b) gathers
    are merged across a super-tile of ST dest tiles (fewer calls -> less
    fixed overhead), (c) nothing else may exceed ~900us.
  - Dest rows are assigned to (core, tile) by a greedy bin-packing that
    equalizes per-(tile, source-block) edge counts across cores (the chunk
    table is shared by all 8 cores).  Output rows are un-permuted on host.
  - DVE one-hot A build pays ~69ns per 128-elem AP row; the output/in0 APs
    are flattened to 2D to (attempt to) amortize it.
  - Epilogue in transposed space: e4T = e1T*(-dscaleT) + xoT fused on DVE,
    bias added via a contract-1 matmul, PSUM->SBUF copies on ACT.
"""

import numpy as np

import concourse.bacc as bacc
import concourse.mybir as mybir
import concourse.tile as tile
from concourse.bass_utils import run_bass_kernel_spmd

FP = mybir.dt.float32
BF = mybir.dt.bfloat16
BF_NP = mybir.dt.np(BF)


class Cfg:
    def __init__(self, n_nodes=100000, n_edges=3200000, f=256, n_cores=8,
                 nb=4, st=4, gather_bufs=8, amat_bufs=2, flat_a=True,
                 rebalance=True):
        self.N = n_nodes
        self.E = n_edges
        self.F = f
        self.NC = n_cores
        self.NB = nb
        self.ST = st
        self.RPC = n_nodes // n_cores
        self.TILES = (self.RPC + 127) // 128
        self.PAD_ROWS = self.TILES * 128
        self.NST = (self.TILES + st - 1) // st
        self.BLK = n_nodes // nb
        assert self.BLK < (1 << 15)
        self.gather_bufs = gather_bufs
        self.amat_bufs = amat_bufs
        self.flat_a = flat_a
        self.rebalance = rebalance


def _assign_rows(cfg, edge_row, edge_col):
    """Greedy bin-packing of dest rows into (core, tile) bins, minimizing
    per-(tile, block) overflow above 8 chunks.  Returns row_perm[N] giving
    the device row ordering: device row (c, t, d) holds original row
    row_perm[c*PAD_ROWS + t*128 + d] (or -1 for unused pad slots)."""
    N, NB, NC, TILES = cfg.N, cfg.NB, cfg.NC, cfg.TILES
    f = np.zeros((N, NB), dtype=np.int64)
    np.add.at(f, (edge_row, edge_col // cfg.BLK), 1)
    if not cfg.rebalance:
        gbin = np.zeros(N, dtype=np.int64)
        for c in range(NC):
            rr = np.arange(c * cfg.RPC, (c + 1) * cfg.RPC)
            gbin[rr] = c * TILES + np.minimum((rr - c * cfg.RPC) // 128,
                                              TILES - 1)
    else:
        tot = f.sum(1)
        nbins = NC * TILES
        capm = np.full((NC, TILES), 128)
        capm[:, -1] = cfg.RPC - (TILES - 1) * 128
        cap = capm.reshape(-1)
        load = np.zeros((nbins, NB), dtype=np.float64)
        cnt = np.zeros(nbins, dtype=np.int64)
        gbin = np.empty(N, dtype=np.int64)
        for r in np.argsort(-tot, kind='stable'):
            nl = load + f[r]
            scores = (np.maximum(nl - 1024.0, 0).sum(axis=1)
                      + nl.max(axis=1) * 1e-3 + (cnt >= cap) * 1e9)
            bb = int(np.argmin(scores))
            gbin[r] = bb
            load[bb] += f[r]
            cnt[bb] += 1

    # row_perm & per-row device slot
    order = np.argsort(gbin, kind='stable')
    row_perm = np.full(NC * cfg.PAD_ROWS, -1, dtype=np.int64)
    row_slot = np.empty(N, dtype=np.int64)  # device row id per original row
    pos_in_bin = np.zeros(NC * TILES, dtype=np.int64)
    gb_sorted = gbin[order]
    # rank within bin
    starts = np.searchsorted(gb_sorted, np.arange(NC * TILES), side='left')
    rank = np.arange(N) - starts[gb_sorted]
    c = gb_sorted // TILES
    t = gb_sorted % TILES
    dev = c * cfg.PAD_ROWS + t * 128 + rank
    row_perm[dev] = order
    row_slot[order] = dev
    return row_perm, row_slot


def _preprocess(cfg, edge_row, edge_col, edge_val, row_slot):
    """Partition + sort + pad the edge list using the device row mapping."""
    edge_row = np.asarray(edge_row).astype(np.int64)
    edge_col = np.asarray(edge_col).astype(np.int64)
    edge_val = np.asarray(edge_val).astype(np.float32)
    NC, TILES, NB, ST, NST, E = cfg.NC, cfg.TILES, cfg.NB, cfg.ST, cfg.NST, cfg.E

    dev = row_slot[edge_row]
    core = dev // cfg.PAD_ROWS
    dloc = dev - core * cfg.PAD_ROWS
    t = dloc >> 7
    d = (dloc & 127).astype(np.float32)
    b = edge_col // cfg.BLK
    cloc = (edge_col - b * cfg.BLK).astype(np.int16)

    # cell order: (super-tile, block, tile-in-st)
    ncell = TILES * NB
    cell_rank = np.zeros((TILES, NB), dtype=np.int64)
    rank = 0
    for stt in range(NST):
        t0, t1 = stt * ST, min((stt + 1) * ST, TILES)
        for bb in range(NB):
            for tt in range(t0, t1):
                cell_rank[tt, bb] = rank
                rank += 1
    assert rank == ncell

    key = core * ncell + cell_rank[t, b]
    order = np.lexsort((cloc, key))
    key_s = key[order]

    counts = np.bincount(key, minlength=NC * ncell).reshape(NC, ncell)
    C = np.ceil(counts.max(axis=0) / 128).astype(np.int64)
    pad_off = np.concatenate([[0], np.cumsum(128 * C)])
    L = int(pad_off[-1])

    starts = np.searchsorted(key_s, np.arange(NC * ncell), side="left")
    rnk = np.arange(E) - starts[key_s]
    pos = (key_s // ncell) * L + pad_off[key_s % ncell] + rnk

    col_pad = np.zeros(NC * L, dtype=np.int16)   # pads gather block row 0
    dest_pad = np.zeros(NC * L, dtype=np.float32)
    val_pad = np.zeros(NC * L, dtype=np.float32)
    col_pad[pos] = cloc[order]
    dest_pad[pos] = d[order]
    val_pad[pos] = edge_val[order]

    col_pad = col_pad.reshape(NC, L)
    dest_pad = dest_pad.reshape(NC, L)
    val_pad = val_pad.reshape(NC, L)

    idx_packed = np.tile(
        col_pad.reshape(NC, L // 16, 16).transpose(0, 2, 1), (1, 8, 1)
    )  # [NC, 128, L//16]

    n_t = np.array([sum(int(C[cell_rank[tt, bb]]) for bb in range(NB))
                    for tt in range(TILES)])
    CT = int(n_t.sum())
    dest_cols = np.zeros((NC, 128, CT), dtype=BF_NP)
    val_cols = np.zeros((NC, 128, CT), dtype=BF_NP)
    tile_coff = np.concatenate([[0], np.cumsum(n_t)])
    for tt in range(TILES):
        toff = int(tile_coff[tt])
        for bb in range(NB):
            r = cell_rank[tt, bb]
            o0, n = int(pad_off[r]), int(C[r])
            if n == 0:
                continue
            seg = slice(o0, o0 + 128 * n)
            dest_cols[:, :, toff:toff + n] = (
                dest_pad[:, seg].reshape(NC, n, 128).transpose(0, 2, 1))
            val_cols[:, :, toff:toff + n] = (
                val_pad[:, seg].reshape(NC, n, 128).transpose(0, 2, 1))
            toff += n

    tables = dict(C=C, cell_rank=cell_rank, pad_off=pad_off, n_t=n_t,
                  tile_coff=tile_coff)
    return tables, idx_packed, dest_cols, val_cols


def _build(cfg, tables):
    F, NB, ST, NST, TILES = cfg.F, cfg.NB, cfg.ST, cfg.NST, cfg.TILES
    C = tables["C"]
    cell_rank = tables["cell_rank"]
    pad_off = tables["pad_off"]
    n_t = tables["n_t"]
    tile_coff = tables["tile_coff"]
    L = int(pad_off[-1])
    C_MAXT = int(n_t.max())
    KC = F // 128

    C_stb = np.zeros((NST, NB), dtype=np.int64)
    gcall_off = np.zeros((NST, NB), dtype=np.int64)
    for stt in range(NST):
        t0, t1 = stt * ST, min((stt + 1) * ST, TILES)
        for bb in range(NB):
            gcall_off[stt, bb] = pad_off[cell_rank[t0, bb]] // 128
            C_stb[stt, bb] = sum(int(C[cell_rank[tt, bb]])
                                 for tt in range(t0, t1))
    C_MAXG = int(C_stb.max())

    nc = bacc.Bacc("TRN2", target_bir_lowering=False, debug=False,
                   num_swdge_queues=4)

    xsrc = nc.dram_tensor("xsrc", [cfg.N, F], BF, kind="ExternalInput")
    xot_d = nc.dram_tensor("xot", [128, TILES, KC, 128], BF,
                           kind="ExternalInput")
    idx_d = nc.dram_tensor("idx", [128, L // 16], mybir.dt.int16,
                           kind="ExternalInput")
    dest_d = nc.dram_tensor("dest", [128, int(n_t.sum())], BF,
                            kind="ExternalInput")
    val_d = nc.dram_tensor("val", [128, int(n_t.sum())], BF,
                           kind="ExternalInput")
    w_d = nc.dram_tensor("wt", [128, KC, F], BF, kind="ExternalInput")
    negds_d = nc.dram_tensor("negds", [128, KC], FP, kind="ExternalInput")
    biasrow_d = nc.dram_tensor("biasrow", [1, F], BF, kind="ExternalInput")
    iota_d = nc.dram_tensor("iota", [128, 128], BF, kind="ExternalInput")
    ident_d = nc.dram_tensor("ident", [128, 128], BF, kind="ExternalInput")
    zeros_d = nc.dram_tensor("zeros", [128, 1], FP, kind="ExternalInput")
    out_d = nc.dram_tensor("out", [cfg.PAD_ROWS, F], BF,
                           kind="ExternalOutput")

    with tile.TileContext(nc) as tc:
        with (
            tc.tile_pool(name="const", bufs=1) as cpool,
            tc.tile_pool(name="gather", bufs=cfg.gather_bufs) as gpool,
            tc.tile_pool(name="amat", bufs=cfg.amat_bufs) as apool,
            tc.tile_pool(name="meta", bufs=8) as mpool,
            tc.tile_pool(name="work", bufs=4) as wpool,
            tc.tile_pool(name="pse1", bufs=2, space="PSUM") as e1pool,
            tc.tile_pool(name="pstr", bufs=2, space="PSUM") as trpool,
            tc.tile_pool(name="psout", bufs=2, space="PSUM") as opool,
        ):
            w_t = cpool.tile([128, KC, F], BF)
            negds_t = cpool.tile([128, KC], FP)
            biasrow_t = cpool.tile([1, F], BF)
            iota_t = cpool.tile([128, 128], BF)
            ident_t = cpool.tile([128, 128], BF)
            zeros_t = cpool.tile([128, 1], FP)
            ones_t = cpool.tile([1, 128], BF)
            nc.sync.dma_start(w_t[:], w_d[:])
            nc.sync.dma_start(negds_t[:], negds_d[:])
            nc.sync.dma_start(biasrow_t[:], biasrow_d[:])
            nc.sync.dma_start(iota_t[:], iota_d[:])
            nc.sync.dma_start(ident_t[:], ident_d[:])
            nc.sync.dma_start(zeros_t[:], zeros_d[:])
            nc.vector.memset(ones_t[:], 1.0)

            iota_rep = cpool.tile([128, C_MAXT, 128], BF)
            nc.vector.tensor_copy(
                iota_rep[:], iota_t[:, None, :].broadcast_to((128, C_MAXT, 128)))

            xgs_by_st = {}
            tile_state = {}
            pending = [None]

            def stage_a(stt, tt):
                """meta DMAs + one-hot A build for tile tt."""
                nt = int(n_t[tt])
                toff = int(tile_coff[tt])
                dest_t = mpool.tile([128, C_MAXT], BF, tag="dest")
                nc.sync.dma_start(dest_t[:, :nt], dest_d[:, toff:toff + nt])
                val_t = mpool.tile([128, C_MAXT], BF, tag="val")
                nc.sync.dma_start(val_t[:, :nt], val_d[:, toff:toff + nt])
                xot = wpool.tile([128, KC, 128], BF, tag="xot")
                nc.sync.dma_start(xot[:], xot_d[:, tt])

                a_t = apool.tile([128, C_MAXT, 128], BF, tag="a")
                dest_b = dest_t[:, :nt, None].broadcast_to((128, nt, 128))
                val_b = val_t[:, :nt, None].broadcast_to((128, nt, 128))
                a_f = a_t.rearrange("p a b -> p (a b)")[:, :nt * 128]
                iota_f = iota_rep.rearrange("p a b -> p (a b)")[:, :nt * 128]
                nc.vector.tensor_tensor(a_f, iota_f, dest_b,
                                        op=mybir.AluOpType.is_equal)
                nc.vector.tensor_tensor(a_f, a_f, val_b,
                                        op=mybir.AluOpType.mult)
                tile_state[tt] = (a_t, xot)

            def stage_b(stt, tt):
                """segment-sum matmuls + epilogue for tile tt."""
                nt = int(n_t[tt])
                a_t, xot = tile_state.pop(tt)
                xgs = xgs_by_st[stt]
                e1 = e1pool.tile([128, F], FP, tag="e1")
                cc = 0
                for bb in range(NB):
                    cb = int(C[cell_rank[tt, bb]])
                    if cb == 0:
                        continue
                    base = int((pad_off[cell_rank[tt, bb]] // 128)
                               - gcall_off[stt, bb])
                    xg = xgs[bb]
                    for c in range(cb):
                        nc.tensor.matmul(
                            e1[:], a_t[:, cc, :], xg[:, base + c, :],
                            start=(cc == 0), stop=(cc == nt - 1),
                        )
                        cc += 1

                e1_sb = wpool.tile([128, F], BF, tag="e1sb")
                nc.scalar.activation(e1_sb[:], e1[:],
                                     mybir.ActivationFunctionType.Identity,
                                     bias=zeros_t[:])
                e1T = trpool.tile([128, KC, 128], BF, tag="tr")
                for kc in range(KC):
                    nc.tensor.transpose(e1T[:, kc, :],
                                        e1_sb[:, kc * 128:(kc + 1) * 128],
                                        ident_t[:])

                e4T = wpool.tile([128, KC, 128], BF, tag="e4T")
                for kc in range(KC):
                    nc.vector.scalar_tensor_tensor(
                        e4T[:, kc, :], e1T[:, kc, :],
                        negds_t[:, kc:kc + 1], xot[:, kc, :],
                        op0=mybir.AluOpType.mult,
                        op1=mybir.AluOpType.add)

                ps_out = opool.tile([128, F], FP, tag="po")
                for kc in range(KC):
                    nc.tensor.matmul(ps_out[:], e4T[:, kc, :], w_t[:, kc, :],
                                     start=(kc == 0), stop=False)
                nc.tensor.matmul(ps_out[:], ones_t[:], biasrow_t[:],
                                 start=False, stop=True)
                outs = wpool.tile([128, F], BF, tag="outs")
                nc.scalar.activation(outs[:], ps_out[:],
                                     mybir.ActivationFunctionType.Identity,
                                     bias=zeros_t[:])
                # out-DMA from ACT's HWDGE queue: keeps the sync queue free
                # for the (latency-critical) idx/meta loads
                nc.scalar.dma_start(out_d[tt * 128:(tt + 1) * 128, :],
                                    outs[:])

            for stt in range(NST):
                t0, t1 = stt * ST, min((stt + 1) * ST, TILES)

                xgs = {}
                for bb in range(NB):
                    cg = int(C_stb[stt, bb])
                    if cg == 0:
                        continue
                    o16 = int(gcall_off[stt, bb]) * 8
                    idx_t = mpool.tile([128, 8 * C_MAXG], mybir.dt.int16,
                                       tag="idx")
                    # only partitions [32b, 32b+32) are read by queue b's
                    # Q7 pair; skip the other 3 replicas
                    nc.sync.dma_start(idx_t[32 * bb:32 * bb + 32, :8 * cg],
                                      idx_d[32 * bb:32 * bb + 32,
                                            o16:o16 + 8 * cg])
                    xg = gpool.tile([128, C_MAXG, F], BF, tag="xg")
                    nc.gpsimd.dma_gather(
                        xg[:, :cg, :],
                        xsrc[bb * cfg.BLK:(bb + 1) * cfg.BLK, :],
                        idx_t[:, :8 * cg],
                        num_idxs=128 * cg,
                        num_idxs_reg=128 * cg,
                        elem_size=F,
                        single_packet=False,
                        queue_num=bb,
                    )
                    xgs[bb] = xg
                xgs_by_st[stt] = xgs

                for tt in range(t0, t1):
                    stage_a(stt, tt)
                    if pending[0] is not None:
                        stage_b(*pending[0])
                    pending[0] = (stt, tt)
            if pending[0] is not None:
                stage_b(*pending[0])

    nc.compile()
    return nc


def _make_in_maps(cfg, x, weight, learnable_diag, bias, row_perm,
                  idx_packed, dest_cols, val_cols):
    F, NC, TILES = cfg.F, cfg.NC, cfg.TILES
    KC = F // 128
    x16 = x.astype(BF_NP)
    w_host = np.ascontiguousarray(
        weight.reshape(KC, 128, F).transpose(1, 0, 2)).astype(BF_NP)
    negds_host = np.ascontiguousarray(
        -(learnable_diag + 1.0).reshape(KC, 128).T).astype(np.float32)
    biasrow_host = bias.reshape(1, F).astype(BF_NP)
    iota_host = np.tile(np.arange(128, dtype=np.float32)[None, :],
                        (128, 1)).astype(BF_NP)
    ident_host = np.eye(128, dtype=np.float32).astype(BF_NP)
    zeros_host = np.zeros((128, 1), dtype=np.float32)

    # residual x rows in device order (permuted), transposed per tile:
    # xot[p, t, kc, d] = xperm[t*128 + d, kc*128 + p]
    xperm = np.zeros((NC * cfg.PAD_ROWS, F), dtype=np.float32)
    valid = row_perm >= 0
    xperm[valid] = x[row_perm[valid]]
    xot_host = np.ascontiguousarray(
        xperm.reshape(NC, TILES, 128, KC, 128).transpose(0, 4, 1, 3, 2)
    ).astype(BF_NP)

    in_maps = []
    for c in range(NC):
        in_maps.append({
            "xsrc": x16,
            "xot": xot_host[c],
            "idx": np.ascontiguousarray(idx_packed[c]),
            "dest": np.ascontiguousarray(dest_cols[c]),
            "val": np.ascontiguousarray(val_cols[c]),
            "wt": w_host,
            "negds": negds_host,
            "biasrow": biasrow_host,
            "iota": iota_host,
            "ident": ident_host,
            "zeros": zeros_host,
        })
    return in_maps


def run(cfg, x, edge_row, edge_col, edge_val, weight, learnable_diag, bias,
        trace_dir=None):
    x = np.ascontiguousarray(np.asarray(x, dtype=np.float32))
    edge_row = np.asarray(edge_row).astype(np.int64)
    edge_col = np.asarray(edge_col).astype(np.int64)
    weight = np.asarray(weight, dtype=np.float32)
    learnable_diag = np.asarray(learnable_diag, dtype=np.float32)
    bias = np.asarray(bias, dtype=np.float32)

    row_perm, row_slot = _assign_rows(cfg, edge_row, edge_col)
    tables, idx_packed, dest_cols, val_cols = _preprocess(
        cfg, edge_row, edge_col, edge_val, row_slot)
    nc = _build(cfg, tables)
    in_maps = _make_in_maps(cfg, x, weight, learnable_diag, bias, row_perm,
                            idx_packed, dest_cols, val_cols)

    kwargs = {}
    if trace_dir:
        kwargs = dict(trace=True, tmpdir=trace_dir)
    res = run_bass_kernel_spmd(nc, in_maps, core_ids=list(range(cfg.NC)),
                               **kwargs)
    out = np.empty((cfg.N, cfg.F), dtype=np.float32)
    for c in range(cfg.NC):
        dev_rows = res.results[c]["out"].astype(np.float32)
        rp = row_perm[c * cfg.PAD_ROWS:(c + 1) * cfg.PAD_ROWS]
        m = rp >= 0
        out[rp[m]] = dev_rows[m]
    return out, res


def kernel(x, edge_row, edge_col, edge_val, weight, learnable_diag, bias,
           _want_trace=None):
    cfg = Cfg()
    out, res = run(cfg, x, edge_row, edge_col, edge_val, weight,
                   learnable_diag, bias, trace_dir=_want_trace)
    kernel._last_results = res
    return out


# revision 12
# speedup vs baseline: 2.0394x; 1.6745x over previous
"""Adagnn-with-weight GNN message-passing kernel for 8 Trainium2 NeuronCores.

Reference computation (N=100000 nodes, E=3200000 edges, F=256):
    e1  = segment_sum(edge_val[:,None] * x[edge_col], edge_row)   # spmm
    out = (x - e1 * (learnable_diag + 1)) @ weight + bias

Architecture (evidence-driven; see traces):
  - Per-edge x rows are gathered by dma_gather (bf16, 512B/edge).  The wall
    is the SW-DMA drain (~250 GB/s for random 512B) and the DVE one-hot
    build; SWDGE desc-gen itself is cheap (~0.5 ns/idx).
  - Host row->pair rebalancing equalizes per-(pair, source-block) edge
    counts across cores (shared chunk table), pad ~5.5%.
  - Dest pairs of 64-row halves: the one-hot A is 64 wide (halves DVE
    work).  Chunks that straddle the half boundary in any core emit two
    A columns (foreign-half vals zeroed on host).  Segment matmuls write
    64-partition PSUM slices of a shared [128,256] e1 accumulator.
  - Gathers merged per super-tile of 4 pairs (one call per source block,
    queue = block); meta loads merged per super-tile to minimize DMA
    instruction count (8 shared HW sem lanes -> reuse fences).
  - Transposed epilogue: e4T = e1T*(-dscaleT) + xoT fused on DVE, bias via
    a contract-1 matmul, PSUM->SBUF copies on ACT, out-DMA on ACT's HWDGE.
"""

import numpy as np

import concourse.bacc as bacc
import concourse.mybir as mybir
import concourse.tile as tile
from concourse.bass_utils import run_bass_kernel_spmd

FP = mybir.dt.float32
BF = mybir.dt.bfloat16
BF_NP = mybir.dt.np(BF)


class Cfg:
    def __init__(self, n_nodes=100000, n_edges=3200000, f=256, n_cores=8,
                 nb=4, st=4, gather_bufs=8, amat_bufs=3, rebalance=True):
        self.N = n_nodes
        self.E = n_edges
        self.F = f
        self.NC = n_cores
        self.NB = nb
        self.ST = st                      # pairs per super-tile
        self.RPC = n_nodes // n_cores
        self.PAIRS = (self.RPC + 127) // 128
        self.PAD_ROWS = self.PAIRS * 128
        self.NST = (self.PAIRS + st - 1) // st
        self.BLK = n_nodes // nb
        assert self.BLK < (1 << 15)
        self.gather_bufs = gather_bufs
        self.amat_bufs = amat_bufs
        self.rebalance = rebalance


def _assign_rows(cfg, edge_row, edge_col):
    """Greedy bin-packing of dest rows into (core, pair) bins minimizing
    per-(pair, block) overflow above 1024 (8 chunks)."""
    N, NB, NC, PAIRS = cfg.N, cfg.NB, cfg.NC, cfg.PAIRS
    f = np.zeros((N, NB), dtype=np.int64)
    np.add.at(f, (edge_row, edge_col // cfg.BLK), 1)
    if not cfg.rebalance:
        gbin = np.zeros(N, dtype=np.int64)
        for c in range(NC):
            rr = np.arange(c * cfg.RPC, (c + 1) * cfg.RPC)
            gbin[rr] = c * PAIRS + np.minimum((rr - c * cfg.RPC) // 128,
                                              PAIRS - 1)
    else:
        tot = f.sum(1)
        nbins = NC * PAIRS
        capm = np.full((NC, PAIRS), 128)
        capm[:, -1] = cfg.RPC - (PAIRS - 1) * 128
        cap = capm.reshape(-1)
        load = np.zeros((nbins, NB), dtype=np.float64)
        cnt = np.zeros(nbins, dtype=np.int64)
        gbin = np.empty(N, dtype=np.int64)
        for r in np.argsort(-tot, kind='stable'):
            nl = load + f[r]
            scores = (np.maximum(nl - 1024.0, 0).sum(axis=1)
                      + nl.max(axis=1) * 1e-3 + (cnt >= cap) * 1e9)
            bb = int(np.argmin(scores))
            gbin[r] = bb
            load[bb] += f[r]
            cnt[bb] += 1

    order = np.argsort(gbin, kind='stable')
    row_perm = np.full(NC * cfg.PAD_ROWS, -1, dtype=np.int64)
    row_slot = np.empty(N, dtype=np.int64)
    gb_sorted = gbin[order]
    starts = np.searchsorted(gb_sorted, np.arange(NC * PAIRS), side='left')
    rank = np.arange(N) - starts[gb_sorted]
    c = gb_sorted // PAIRS
    t = gb_sorted % PAIRS
    dev = c * cfg.PAD_ROWS + t * 128 + rank
    row_perm[dev] = order
    row_slot[order] = dev
    return row_perm, row_slot


def _preprocess(cfg, edge_row, edge_col, edge_val, row_slot):
    """Partition + sort + pad the edge list; build per-column one-hot
    metadata (64-wide halves with union straddle columns)."""
    edge_row = np.asarray(edge_row).astype(np.int64)
    edge_col = np.asarray(edge_col).astype(np.int64)
    edge_val = np.asarray(edge_val).astype(np.float32)
    NC, PAIRS, NB, ST, NST, E = (cfg.NC, cfg.PAIRS, cfg.NB, cfg.ST, cfg.NST,
                                 cfg.E)

    dev = row_slot[edge_row]
    core = dev // cfg.PAD_ROWS
    dloc = dev - core * cfg.PAD_ROWS
    t = dloc >> 7                       # pair index
    half = (dloc >> 6) & 1
    d64 = (dloc & 63).astype(np.float32)
    b = edge_col // cfg.BLK
    cloc = (edge_col - b * cfg.BLK).astype(np.int16)

    # cell order: (super-tile, block, pair-in-st)
    ncell = PAIRS * NB
    cell_rank = np.zeros((PAIRS, NB), dtype=np.int64)
    rank = 0
    for stt in range(NST):
        t0, t1 = stt * ST, min((stt + 1) * ST, PAIRS)
        for bb in range(NB):
            for tt in range(t0, t1):
                cell_rank[tt, bb] = rank
                rank += 1
    assert rank == ncell

    key = core * ncell + cell_rank[t, b]
    order = np.lexsort((cloc, half, key))
    key_s = key[order]

    counts = np.bincount(key, minlength=NC * ncell).reshape(NC, ncell)
    C = np.ceil(counts.max(axis=0) / 128).astype(np.int64)
    pad_off = np.concatenate([[0], np.cumsum(128 * C)])
    L = int(pad_off[-1])

    starts = np.searchsorted(key_s, np.arange(NC * ncell), side="left")
    rnk = np.arange(E) - starts[key_s]
    pos = (key_s // ncell) * L + pad_off[key_s % ncell] + rnk

    col_pad = np.zeros(NC * L, dtype=np.int16)     # pads gather block row 0
    dest_pad = np.zeros(NC * L, dtype=np.float32)
    half_pad = np.full(NC * L, -1, dtype=np.int8)
    val_pad = np.zeros(NC * L, dtype=np.float32)
    col_pad[pos] = cloc[order]
    dest_pad[pos] = d64[order]
    half_pad[pos] = half[order]
    val_pad[pos] = edge_val[order]

    col_pad = col_pad.reshape(NC, L)
    dest_pad = dest_pad.reshape(NC, L)
    half_pad = half_pad.reshape(NC, L)
    val_pad = val_pad.reshape(NC, L)

    idx_packed = np.tile(
        col_pad.reshape(NC, L // 16, 16).transpose(0, 2, 1), (1, 8, 1)
    )  # [NC, 128, L//16]

    # per-chunk halves present (union over cores)
    nchunk = L // 128
    hp = half_pad.reshape(NC, nchunk, 128)
    has0 = (hp == 0).any(axis=(0, 2))
    has1 = (hp == 1).any(axis=(0, 2))

    # per-pair column lists: (half, global chunk)
    cols_per_tile = []
    for tt in range(PAIRS):
        cols = []
        for bb in range(NB):
            r = cell_rank[tt, bb]
            k0 = int(pad_off[r]) // 128
            for k in range(k0, k0 + int(C[r])):
                if has0[k]:
                    cols.append((0, k))
                if has1[k]:
                    cols.append((1, k))
        cols_per_tile.append(cols)
    n_cols = np.array([len(c) for c in cols_per_tile])
    CTC = int(n_cols.sum())
    col_off = np.concatenate([[0], np.cumsum(n_cols)])

    dest_cols = np.zeros((NC, 128, CTC), dtype=BF_NP)
    val_cols = np.zeros((NC, 128, CTC), dtype=BF_NP)
    dpc = dest_pad.reshape(NC, nchunk, 128)
    vpc = val_pad.reshape(NC, nchunk, 128)
    j = 0
    for tt in range(PAIRS):
        for (h, k) in cols_per_tile[tt]:
            dest_cols[:, :, j] = dpc[:, k, :]
            val_cols[:, :, j] = np.where(hp[:, k, :] == h, vpc[:, k, :], 0.0)
            j += 1
    assert j == CTC

    tables = dict(C=C, cell_rank=cell_rank, pad_off=pad_off,
                  cols_per_tile=cols_per_tile, n_cols=n_cols, col_off=col_off)
    return tables, idx_packed, dest_cols, val_cols


def _build(cfg, tables):
    F, NB, ST, NST, PAIRS = cfg.F, cfg.NB, cfg.ST, cfg.NST, cfg.PAIRS
    C = tables["C"]
    cell_rank = tables["cell_rank"]
    pad_off = tables["pad_off"]
    cols_per_tile = tables["cols_per_tile"]
    n_cols = tables["n_cols"]
    col_off = tables["col_off"]
    L = int(pad_off[-1])
    CTC = int(n_cols.sum())
    NCMAX = int(n_cols.max())
    KC = F // 128

    C_stb = np.zeros((NST, NB), dtype=np.int64)
    gcall_off = np.zeros((NST, NB), dtype=np.int64)
    st_cols = np.zeros(NST, dtype=np.int64)      # columns per super-tile
    for stt in range(NST):
        t0, t1 = stt * ST, min((stt + 1) * ST, PAIRS)
        for bb in range(NB):
            gcall_off[stt, bb] = pad_off[cell_rank[t0, bb]] // 128
            C_stb[stt, bb] = sum(int(C[cell_rank[tt, bb]])
                                 for tt in range(t0, t1))
        st_cols[stt] = sum(int(n_cols[tt]) for tt in range(t0, t1))
    C_MAXG = int(C_stb.max())
    STC_MAX = int(st_cols.max())

    nc = bacc.Bacc("TRN2", target_bir_lowering=False, debug=False,
                   num_swdge_queues=4)

    xsrc = nc.dram_tensor("xsrc", [cfg.N, F], BF, kind="ExternalInput")
    xot_d = nc.dram_tensor("xot", [128, PAIRS, KC, 128], BF,
                           kind="ExternalInput")
    idx_d = nc.dram_tensor("idx", [128, L // 16], mybir.dt.int16,
                           kind="ExternalInput")
    dest_d = nc.dram_tensor("dest", [128, CTC], BF, kind="ExternalInput")
    val_d = nc.dram_tensor("val", [128, CTC], BF, kind="ExternalInput")
    w_d = nc.dram_tensor("wt", [128, KC, F], BF, kind="ExternalInput")
    negds_d = nc.dram_tensor("negds", [128, KC], FP, kind="ExternalInput")
    biasrow_d = nc.dram_tensor("biasrow", [1, F], BF, kind="ExternalInput")
    iota_d = nc.dram_tensor("iota", [128, 64], BF, kind="ExternalInput")
    ident_d = nc.dram_tensor("ident", [128, 128], BF, kind="ExternalInput")
    zeros_d = nc.dram_tensor("zeros", [128, 1], FP, kind="ExternalInput")
    out_d = nc.dram_tensor("out", [cfg.PAD_ROWS, F], BF,
                           kind="ExternalOutput")

    with tile.TileContext(nc) as tc:
        with (
            tc.tile_pool(name="const", bufs=1) as cpool,
            tc.tile_pool(name="gather", bufs=cfg.gather_bufs) as gpool,
            tc.tile_pool(name="amat", bufs=cfg.amat_bufs) as apool,
            tc.tile_pool(name="meta", bufs=8) as mpool,
            tc.tile_pool(name="stmeta", bufs=3) as spool,
            tc.tile_pool(name="work", bufs=4) as wpool,
            tc.tile_pool(name="pse1", bufs=2, space="PSUM") as e1pool,
            tc.tile_pool(name="pstr", bufs=2, space="PSUM") as trpool,
            tc.tile_pool(name="psout", bufs=2, space="PSUM") as opool,
        ):
            w_t = cpool.tile([128, KC, F], BF)
            negds_t = cpool.tile([128, KC], FP)
            biasrow_t = cpool.tile([1, F], BF)
            iota_t = cpool.tile([128, 64], BF)
            ident_t = cpool.tile([128, 128], BF)
            zeros_t = cpool.tile([128, 1], FP)
            ones_t = cpool.tile([1, 128], BF)
            nc.sync.dma_start(w_t[:], w_d[:])
            nc.sync.dma_start(negds_t[:], negds_d[:])
            nc.sync.dma_start(biasrow_t[:], biasrow_d[:])
            nc.sync.dma_start(iota_t[:], iota_d[:])
            nc.sync.dma_start(ident_t[:], ident_d[:])
            nc.sync.dma_start(zeros_t[:], zeros_d[:])
            nc.vector.memset(ones_t[:], 1.0)

            iota_rep = cpool.tile([128, NCMAX, 64], BF)
            nc.vector.tensor_copy(
                iota_rep[:], iota_t[:, None, :].broadcast_to((128, NCMAX, 64)))

            xgs_by_st = {}
            st_meta = {}
            tile_state = {}
            pending = [None]

            def stage_a(stt, tt):
                """one-hot A build for pair tt (64-wide columns)."""
                ncl = int(n_cols[tt])
                dest_st, val_st, xot_st = st_meta[stt]
                loc0 = int(col_off[tt] - col_off[stt * ST])
                a_t = apool.tile([128, NCMAX, 64], BF, tag="a")
                dest_b = dest_st[:, loc0:loc0 + ncl, None].broadcast_to(
                    (128, ncl, 64))
                val_b = val_st[:, loc0:loc0 + ncl, None].broadcast_to(
                    (128, ncl, 64))
                a_f = a_t.rearrange("p a b -> p (a b)")[:, :ncl * 64]
                iota_f = iota_rep.rearrange("p a b -> p (a b)")[:, :ncl * 64]
                nc.vector.tensor_tensor(a_f, iota_f, dest_b,
                                        op=mybir.AluOpType.is_equal)
                nc.vector.tensor_tensor(a_f, a_f, val_b,
                                        op=mybir.AluOpType.mult)
                tile_state[tt] = (a_t, xot_st)

            def stage_b(stt, tt):
                """segment-sum matmuls + epilogue for pair tt."""
                a_t, xot_st = tile_state.pop(tt)
                xgs = xgs_by_st[stt]
                cols = cols_per_tile[tt]
                # chunk -> (block, local xg index)
                k2loc = {}
                for bb in range(NB):
                    r = cell_rank[tt, bb]
                    k0 = int(pad_off[r]) // 128
                    for k in range(k0, k0 + int(C[r])):
                        k2loc[k] = (bb, k - int(gcall_off[stt, bb]))
                # start/stop per half
                first = {0: None, 1: None}
                last = {0: None, 1: None}
                for j, (h, k) in enumerate(cols):
                    if first[h] is None:
                        first[h] = j
                    last[h] = j
                e1 = e1pool.tile([128, F], FP, tag="e1")
                for j, (h, k) in enumerate(cols):
                    bb, loc = k2loc[k]
                    nc.tensor.matmul(
                        e1[64 * h:64 * h + 64, :], a_t[:, j, :],
                        xgs[bb][:, loc, :],
                        start=(j == first[h]), stop=(j == last[h]),
                    )

                e1_sb = wpool.tile([128, F], BF, tag="e1sb")
                nc.scalar.activation(e1_sb[:], e1[:],
                                     mybir.ActivationFunctionType.Identity,
                                     bias=zeros_t[:])
                e1T = trpool.tile([128, KC, 128], BF, tag="tr")
                for kc in range(KC):
                    nc.tensor.transpose(e1T[:, kc, :],
                                        e1_sb[:, kc * 128:(kc + 1) * 128],
                                        ident_t[:])

                e4T = wpool.tile([128, KC, 128], BF, tag="e4T")
                for kc in range(KC):
                    nc.vector.scalar_tensor_tensor(
                        e4T[:, kc, :], e1T[:, kc, :],
                        negds_t[:, kc:kc + 1],
                        xot_st[:, tt - stt * ST, kc, :],
                        op0=mybir.AluOpType.mult,
                        op1=mybir.AluOpType.add)

                ps_out = opool.tile([128, F], FP, tag="po")
                for kc in range(KC):
                    nc.tensor.matmul(ps_out[:], e4T[:, kc, :], w_t[:, kc, :],
                                     start=(kc == 0), stop=False)
                nc.tensor.matmul(ps_out[:], ones_t[:], biasrow_t[:],
                                 start=False, stop=True)
                outs = wpool.tile([128, F], BF, tag="outs")
                nc.scalar.activation(outs[:], ps_out[:],
                                     mybir.ActivationFunctionType.Identity,
                                     bias=zeros_t[:])
                nc.scalar.dma_start(out_d[tt * 128:(tt + 1) * 128, :],
                                    outs[:])

            for stt in range(NST):
                t0, t1 = stt * ST, min((stt + 1) * ST, PAIRS)

                # merged meta loads for the super-tile
                scl = int(st_cols[stt])
                co0 = int(col_off[t0])
                dest_st = spool.tile([128, STC_MAX], BF, tag="dest")
                nc.sync.dma_start(dest_st[:, :scl], dest_d[:, co0:co0 + scl])
                val_st = spool.tile([128, STC_MAX], BF, tag="val")
                nc.sync.dma_start(val_st[:, :scl], val_d[:, co0:co0 + scl])
                xot_st = spool.tile([128, ST, KC, 128], BF, tag="xot")
                nc.sync.dma_start(xot_st[:, :t1 - t0], xot_d[:, t0:t1])
                st_meta[stt] = (dest_st, val_st, xot_st)

                xgs = {}
                for bb in range(NB):
                    cg = int(C_stb[stt, bb])
                    if cg == 0:
                        continue
                    o16 = int(gcall_off[stt, bb]) * 8
                    idx_t = mpool.tile([128, 8 * C_MAXG], mybir.dt.int16,
                                       tag="idx")
                    # only partitions [32b, 32b+32) are read by queue b's
                    # Q7 pair; skip the other 3 replicas
                    nc.sync.dma_start(idx_t[32 * bb:32 * bb + 32, :8 * cg],
                                      idx_d[32 * bb:32 * bb + 32,
                                            o16:o16 + 8 * cg])
                    xg = gpool.tile([128, C_MAXG, F], BF, tag="xg")
                    nc.gpsimd.dma_gather(
                        xg[:, :cg, :],
                        xsrc[bb * cfg.BLK:(bb + 1) * cfg.BLK, :],
                        idx_t[:, :8 * cg],
                        num_idxs=128 * cg,
                        num_idxs_reg=128 * cg,
                        elem_size=F,
                        single_packet=False,
                        queue_num=bb,
                    )
                    xgs[bb] = xg
                xgs_by_st[stt] = xgs

                for tt in range(t0, t1):
                    stage_a(stt, tt)
                    if pending[0] is not None:
                        stage_b(*pending[0])
                    pending[0] = (stt, tt)
            if pending[0] is not None:
                stage_b(*pending[0])

    nc.compile()
    return nc


def _make_in_maps(cfg, x, weight, learnable_diag, bias, row_perm,
                  idx_packed, dest_cols, val_cols):
    F, NC, PAIRS = cfg.F, cfg.NC, cfg.PAIRS
    KC = F // 128
    x16 = x.astype(BF_NP)
    w_host = np.ascontiguousarray(
        weight.reshape(KC, 128, F).transpose(1, 0, 2)).astype(BF_NP)
    negds_host = np.ascontiguousarray(
        -(learnable_diag + 1.0).reshape(KC, 128).T).astype(np.float32)
    biasrow_host = bias.reshape(1, F).astype(BF_NP)
    iota_host = np.tile(np.arange(64, dtype=np.float32)[None, :],
                        (128, 1)).astype(BF_NP)
    ident_host = np.eye(128, dtype=np.float32).astype(BF_NP)
    zeros_host = np.zeros((128, 1), dtype=np.float32)

    xperm = np.zeros((NC * cfg.PAD_ROWS, F), dtype=np.float32)
    valid = row_perm >= 0
    xperm[valid] = x[row_perm[valid]]
    xot_host = np.ascontiguousarray(
        xperm.reshape(NC, PAIRS, 128, KC, 128).transpose(0, 4, 1, 3, 2)
    ).astype(BF_NP)

    in_maps = []
    for c in range(NC):
        in_maps.append({
            "xsrc": x16,
            "xot": xot_host[c],
            "idx": np.ascontiguousarray(idx_packed[c]),
            "dest": np.ascontiguousarray(dest_cols[c]),
            "val": np.ascontiguousarray(val_cols[c]),
            "wt": w_host,
            "negds": negds_host,
            "biasrow": biasrow_host,
            "iota": iota_host,
            "ident": ident_host,
            "zeros": zeros_host,
        })
    return in_maps


def run(cfg, x, edge_row, edge_col, edge_val, weight, learnable_diag, bias,
        trace_dir=None):
    x = np.ascontiguousarray(np.asarray(x, dtype=np.float32))
    edge_row = np.asarray(edge_row).astype(np.int64)
    edge_col = np.asarray(edge_col).astype(np.int64)
    weight = np.asarray(weight, dtype=np.float32)
    learnable_diag = np.asarray(learnable_diag, dtype=np.float32)
    bias = np.asarray(bias, dtype=np.float32)

    row_perm, row_slot = _assign_rows(cfg, edge_row, edge_col)
    tables, idx_packed, dest_cols, val_cols = _preprocess(
        cfg, edge_row, edge_col, edge_val, row_slot)
    nc = _build(cfg, tables)
    in_maps = _make_in_maps(cfg, x, weight, learnable_diag, bias, row_perm,
                            idx_packed, dest_cols, val_cols)

    kwargs = {}
    if trace_dir:
        kwargs = dict(trace=True, tmpdir=trace_dir)
    res = run_bass_kernel_spmd(nc, in_maps, core_ids=list(range(cfg.NC)),
                               **kwargs)
    out = np.empty((cfg.N, cfg.F), dtype=np.float32)
    for c in range(cfg.NC):
        dev_rows = res.results[c]["out"].astype(np.float32)
        rp = row_perm[c * cfg.PAD_ROWS:(c + 1) * cfg.PAD_ROWS]
        m = rp >= 0
        out[rp[m]] = dev_rows[m]
    return out, res


def kernel(x, edge_row, edge_col, edge_val, weight, learnable_diag, bias,
           _want_trace=None):
    cfg = Cfg()
    out, res = run(cfg, x, edge_row, edge_col, edge_val, weight,
                   learnable_diag, bias, trace_dir=_want_trace)
    kernel._last_results = res
    return out
